# revision 13
# baseline (speedup 1.0000x reference)
"""Trainium2 Bass kernel for nn_DiffusionLayer (gnn_message_passing).

Computation (full shapes):
  x (16,64,64,512), A (16,512,64,64), phys_prior (16,64,512) ->
  corr (16,32,64,512)

Sharding: pure data parallel over batch B=16 across 8 cores (B_LOC=2 each).
All reductions are local to a (b, m) tile; scalar params replicated.

Per-core design (v2 — contiguous-A):
  A is loaded CONTIGUOUSLY as [p=m (128/tile), f=(c,d)=4096] so every
  partition reads one 16 KiB HBM run (line rate; the old (m0,c)-partition
  layout had 256 B runs -> ~1.6x DMA penalty, and its per-4-m fp32
  LDWEIGHTS burned ~120 us of PE).

  deg[m,c] = sum_d A[m,c,d]   : DVE tensor_reduce over innermost d.
  As[m,d]  = sum_c A[m,c,d]*sT[m,c] : gpsimd broadcast-multiply
             (stride-0 AP on sT over d) into prod, then DVE strided
             reduce over c (view [p, d, c]).  Split into two c-halves
             per tile so the reduce of half 0 overlaps the multiply of
             half 1.
  sT[m,c] (s transposed) via PE transposes of s_t chunks -> PSUM -> SBUF.
  deg_m/As_m [m,c-ish] are PE-transposed into PSUM tiles deg_ps/as_ps
  [p=(b,c), m] that feed the combine stage directly.

  s = mean_f x : PE blockdiag-ones matmul (as before), per b.
  combine + 1x1-conv output stages: unchanged from baseline (full-width
  [128=(b,c), m-quarter] DVE/ACT ops; out DMA 512 B runs at line rate).

DMA schedule: sync ring = x_b0(half), A_b0, A_b1(mt0,mt2); scalar ring =
consts, pp, x_b0(half), x_b1, A_b1(mt1,mt3), out.  x_b0 gets both rings
first (s_b0 gates the gpsimd product stream), A_b0 follows immediately,
x_b1 rides the scalar ring so sT_b1 is ready before the b1 product pass.
"""

import sys
import numpy as np

sys.path.insert(0, "/opt/trn_rl_repo")

import concourse.bass as bass  # noqa: E402
from concourse import bacc  # noqa: E402
import concourse.tile as tile  # noqa: E402
from concourse import mybir  # noqa: E402
from concourse.bass import broadcast_tensor_aps  # noqa: E402
from concourse.bass_utils import run_bass_kernel_spmd  # noqa: E402

B, F_DIM, C, M = 16, 64, 64, 512
OUT_CH = 32
DT = 1.0
N_CORES = 8
B_LOC = B // N_CORES  # 2
F32 = mybir.dt.float32
M_T = 128  # m's per A tile (= one quarter)
NMT = M // M_T  # 4 A tiles per b

_CACHE = {}


def _build_bass():
    nc = bacc.Bacc()

    x_sh = nc.declare_dram_parameter("x_sh", [B_LOC, F_DIM, C, M], F32, isOutput=False)
    a_sh = nc.declare_dram_parameter("a_sh", [B_LOC, M, C, C], F32, isOutput=False)
    pp_sh = nc.declare_dram_parameter("pp_sh", [B_LOC, C, M], F32, isOutput=False)
    ones_bd = nc.declare_dram_parameter("ones_bd", [128, C], F32, isOutput=False)
    w1r = nc.declare_dram_parameter("w1r", [128, 16], F32, isOutput=False)
    b1r = nc.declare_dram_parameter("b1r", [128, 16], F32, isOutput=False)
    w2r = nc.declare_dram_parameter("w2r", [128, 16], F32, isOutput=False)
    cvec = nc.declare_dram_parameter("cvec", [128, 4], F32, isOutput=False)
    pwpb = nc.declare_dram_parameter("pwpb", [128, 2 * OUT_CH], F32, isOutput=False)
    idn = nc.declare_dram_parameter("idn", [128, 128], F32, isOutput=False)
    out_sh = nc.declare_dram_parameter("out", [B_LOC, OUT_CH, C, M], F32, isOutput=True)

    AX = mybir.AxisListType
    OP = mybir.AluOpType
    ACTF = mybir.ActivationFunctionType

    with tile.TileContext(nc) as tc:
        with (
            tc.tile_pool(name="const", bufs=1) as cpool,
            tc.tile_pool(name="xp", bufs=4) as xpool,
            tc.tile_pool(name="ap", bufs=4) as apool,
            tc.tile_pool(name="pp", bufs=3) as prpool,
            tc.tile_pool(name="sp", bufs=1) as spool,
            tc.tile_pool(name="dm", bufs=4) as dmpool,
            tc.tile_pool(name="tmp", bufs=2) as tpool,
            tc.tile_pool(name="small", bufs=1) as smpool,
            tc.tile_pool(name="op", bufs=5) as opool,
            tc.tile_pool(name="ps_s", bufs=1, space="PSUM") as ps_s_pool,
            tc.tile_pool(name="ps_st", bufs=1, space="PSUM") as ps_st_pool,
            tc.tile_pool(name="ps_deg", bufs=1, space="PSUM") as ps_deg_pool,
            tc.tile_pool(name="ps_as", bufs=1, space="PSUM") as ps_as_pool,
        ):
            # ---- constants on the scalar ring ----
            NCC = C + 16 * 3 + 4 + 2 * OUT_CH
            call_t = cpool.tile([128, NCC], F32)
            nc.scalar.dma_start(call_t[:, 0:C], ones_bd[:])
            nc.scalar.dma_start(call_t[:, C : C + 16], w1r[:])
            nc.scalar.dma_start(call_t[:, C + 16 : C + 32], b1r[:])
            nc.scalar.dma_start(call_t[:, C + 32 : C + 48], w2r[:])
            nc.scalar.dma_start(call_t[:, C + 48 : C + 52], cvec[:])
            nc.scalar.dma_start(call_t[:, C + 52 : NCC], pwpb[:])
            ones_t = call_t[:, 0:C]
            w1r_t = call_t[:, C : C + 16]
            b1r_t = call_t[:, C + 16 : C + 32]
            w2r_t = call_t[:, C + 32 : C + 48]
            cvec_t = call_t[:, C + 48 : C + 52]
            pwpb_t = call_t[:, C + 52 : NCC]
            idn_t = cpool.tile([128, 128], F32)
            nc.scalar.dma_start(idn_t[:], idn[:])
            pp_t = spool.tile([128, M], F32)

            s_ps = ps_s_pool.tile([128, M], F32)
            s_t = spool.tile([128, M], F32)
            sT_ps = ps_st_pool.tile([128, 2 * NMT * C], F32)  # [128, 512]
            sT_sb = spool.tile([128, 2 * NMT * C], F32)
            deg_ps = ps_deg_pool.tile([128, M], F32)
            as_ps = ps_as_pool.tile([128, M], F32)
            snew = spool.tile([128, M], F32)

            NFP = F_DIM // 2  # 32 f-pairs per b
            FPG = 8  # f-pairs per DMA -> 4 x-tiles of 2 MiB per b
            NXG = NFP // FPG  # 4

            x_tiles = {}

            def emit_x_dma(b):
                for g in range(NXG):
                    xt = xpool.tile([128, FPG * M], F32, tag="xt")
                    xin = x_sh[b, 2 * g * FPG : 2 * (g + 1) * FPG].rearrange(
                        "(fp ftwo) c m -> ftwo c fp m", ftwo=2
                    )
                    if b == 0:
                        xeng = nc.sync if g < 2 else nc.scalar
                    else:
                        xeng = nc.scalar if g < 2 else nc.sync
                    xeng.dma_start(xt[:].rearrange("p (fp m) -> p fp m", m=M), xin)
                    x_tiles[(b, g)] = xt

            def emit_x_mm(b, g):
                xt = x_tiles.pop((b, g))
                for j in range(FPG):
                    fp = g * FPG + j
                    nc.tensor.matmul(
                        s_ps[b * C : (b + 1) * C, :],
                        ones_t[:],
                        xt[:, j * M : (j + 1) * M],
                        start=(fp == 0),
                        stop=(fp == NFP - 1),
                    )

            def emit_s_finish(b):
                # s_t = psum/F ; sT chunks via PE transpose -> psum -> SBUF.
                # Copies ride DVE: the ACT queue dispatches ring DMAs and can
                # jam behind their WAR waits.
                bsl = slice(b * C, (b + 1) * C)
                nc.vector.tensor_scalar_mul(s_t[bsl, :], s_ps[bsl, :], 1.0 / F_DIM)
                for ch in range(NMT):
                    co = b * NMT * C + ch * C
                    nc.tensor.transpose(
                        sT_ps[:, co : co + C],
                        s_t[bsl, ch * M_T : (ch + 1) * M_T],
                        idn_t[bsl, b * C : (b + 1) * C],
                    )
                half = slice(b * NMT * C, (b + 1) * NMT * C)
                nc.vector.tensor_copy(sT_sb[:, half], sT_ps[:, half])

            rdt_box = {}

            def emit_mlp():
                rsum = smpool.tile([128, 1], F32)
                nc.vector.tensor_reduce(rsum[:], s_t[:], axis=AX.X, op=OP.add)
                rin = smpool.tile([128, 1], F32)
                nc.vector.tensor_scalar_mul(rin[:], rsum[:], 1.0 / M)
                hp = smpool.tile([128, 16], F32)
                nc.vector.tensor_scalar(hp[:], w1r_t[:], rin[:], None, op0=OP.mult)
                nc.vector.tensor_add(hp[:], hp[:], b1r_t[:])
                hneg = smpool.tile([128, 16], F32)
                nc.vector.tensor_scalar_min(hneg[:], hp[:], 0.0)
                hexp = smpool.tile([128, 16], F32)
                nc.scalar.activation(hexp[:], hneg[:], ACTF.Exp)
                hrelu = smpool.tile([128, 16], F32)
                nc.vector.tensor_scalar_max(hrelu[:], hp[:], 0.0)
                helu = smpool.tile([128, 16], F32)
                nc.vector.tensor_add(helu[:], hexp[:], hrelu[:])
                # helu = elu + 1 ; host folds the -1 via cvec[:,3]
                hw = smpool.tile([128, 16], F32)
                nc.vector.tensor_mul(hw[:], helu[:], w2r_t[:])
                rpre = smpool.tile([128, 1], F32)
                nc.vector.tensor_reduce(rpre[:], hw[:], axis=AX.X, op=OP.add)
                rdt = smpool.tile([128, 1], F32)
                nc.vector.tensor_scalar(
                    rdt[:], rpre[:], cvec_t[:, 3:4], None, op0=OP.add
                )
                rdt_box["rdt"] = rdt

            CH = C // 2  # c-half = 32
            HW = CH * C  # flat cols per c-half = 2048

            # per-mt [p=m, f=(b,c)] staging tiles; one 128x128 transpose per
            # mt moves BOTH b's halves to [(b,c), m] psum (verifier requires
            # transpose outputs at psum partition 0).
            degm_tiles = {}
            asm_tiles = {}

            def emit_a_tile(b, mt, eng):
                """Stream one A tile and produce deg/As halves."""
                at = apool.tile([128, C * C], F32, tag="at")
                ain = a_sh[b, mt * M_T : (mt + 1) * M_T].rearrange("m c d -> m (c d)")
                eng.dma_start(at[:], ain)
                if mt not in degm_tiles:
                    degm_tiles[mt] = dmpool.tile(
                        [128, 2 * C], F32, tag=f"degm{mt}", name=f"degm{mt}"
                    )
                    asm_tiles[mt] = dmpool.tile(
                        [128, 2 * C], F32, tag=f"asm{mt}", name=f"asm{mt}"
                    )
                degm = degm_tiles[mt]
                asm = asm_tiles[mt]
                bh = slice(b * C, (b + 1) * C)  # column half for this b
                # deg_m[m, c] = sum_d A[m, c, d]
                nc.vector.tensor_reduce(
                    degm[:, bh], at[:].rearrange("p (c d) -> p c d", d=C),
                    axis=AX.X, op=OP.add,
                )
                # As: two c-halves; gpsimd broadcast multiply, DVE strided reduce
                co = b * NMT * C + mt * C
                asp = []
                for h in range(2):
                    prod = prpool.tile([128, HW], F32, tag="prod")
                    a_v = at[:, h * HW : (h + 1) * HW].rearrange(
                        "p (c d) -> p c d", d=C
                    )
                    st_v = sT_sb[:, co + h * CH : co + (h + 1) * CH].rearrange(
                        "p (c one) -> p c one", one=1
                    )
                    a_bv, st_bv = broadcast_tensor_aps(a_v, st_v)
                    # gpsimd walks APs in software: write prod TRANSPOSED
                    # ((d, c) layout) so the DVE reduce below is contiguous
                    # (strided reduce measured ~2x slower).
                    nc.gpsimd.tensor_tensor(
                        prod[:].rearrange("p (d c) -> p c d", c=CH),
                        a_bv, st_bv, op=OP.mult,
                    )
                    ph = dmpool.tile([128, C], F32, tag=f"asp{h}")
                    nc.vector.tensor_reduce(
                        ph[:], prod[:].rearrange("p (d c) -> p d c", c=CH),
                        axis=AX.X, op=OP.add,
                    )
                    asp.append(ph)
                nc.vector.tensor_add(asm[:, bh], asp[0][:], asp[1][:])

            def emit_degas_transpose(mt):
                nc.tensor.transpose(
                    deg_ps[:, mt * M_T : (mt + 1) * M_T],
                    degm_tiles.pop(mt)[:], idn_t[:],
                )
                nc.tensor.transpose(
                    as_ps[:, mt * M_T : (mt + 1) * M_T],
                    asm_tiles.pop(mt)[:], idn_t[:],
                )

            OG = 8  # out channels per DMA

            def emit_combine_out(q):
                hs = slice(q * M_T, (q + 1) * M_T)
                t2p = tpool.tile([128, M_T], F32, tag="t2p")
                nc.vector.tensor_scalar(
                    t2p[:], deg_ps[:, hs], cvec_t[:, 0:1], 1.0,
                    op0=OP.mult, op1=OP.add,
                )
                t2 = tpool.tile([128, M_T], F32, tag="t2")
                nc.vector.tensor_mul(t2[:], t2p[:], s_t[:, hs])
                t3 = tpool.tile([128, M_T], F32, tag="t3")
                nc.vector.tensor_scalar(
                    t3[:], as_ps[:, hs], cvec_t[:, 1:2], None, op0=OP.mult
                )
                t4 = tpool.tile([128, M_T], F32, tag="t4")
                nc.vector.tensor_add(t4[:], t2[:], t3[:])
                t5 = tpool.tile([128, M_T], F32, tag="t5")
                nc.vector.tensor_scalar(
                    t5[:], pp_t[:, hs], cvec_t[:, 2:3], rdt_box["rdt"][:],
                    op0=OP.mult, op1=OP.add,
                )
                nc.vector.tensor_add(snew[:, hs], t4[:], t5[:])
                for og in range(OUT_CH // OG):
                    ot = opool.tile([128, OG * M_T], F32, tag="ot")
                    for g in range(OG):
                        o = og * OG + g
                        if g % 2 == 0:
                            nc.vector.tensor_scalar(
                                ot[:, g * M_T : (g + 1) * M_T],
                                snew[:, hs],
                                pwpb_t[:, 2 * o : 2 * o + 1],
                                pwpb_t[:, 2 * o + 1 : 2 * o + 2],
                                op0=OP.mult, op1=OP.add,
                            )
                        else:
                            nc.scalar.activation(
                                ot[:, g * M_T : (g + 1) * M_T],
                                snew[:, hs],
                                ACTF.Identity,
                                bias=pwpb_t[:, 2 * o + 1 : 2 * o + 2],
                                scale=pwpb_t[:, 2 * o : 2 * o + 1],
                            )
                    for b in range(B_LOC):
                        odst = out_sh[
                            b, og * OG : (og + 1) * OG, :, q * M_T : (q + 1) * M_T
                        ].rearrange("o c m -> c o m")
                        osrc = ot[b * C : (b + 1) * C, :].rearrange(
                            "p (o m) -> p o m", m=M_T
                        )
                        nc.scalar.dma_start(odst, osrc)

            # ---------------- emission order ----------------
            # Rings: sync = xb0(g0,g1), A_b0, xb1(g2,g3);
            #        scalar = consts, xb0(g2,g3), xb1(g0,g1), pp, A_b1, out.
            emit_x_dma(0)
            for g in range(NXG):
                emit_x_mm(0, g)
            emit_s_finish(0)
            emit_x_dma(1)
            nc.scalar.dma_start(pp_t[:], pp_sh[:])
            # A_b0 stream, with s_b1 matmul groups interleaved on PE
            for mt in range(NMT):
                emit_a_tile(0, mt, nc.sync)
                emit_x_mm(1, mt)
            emit_s_finish(1)
            emit_mlp()
            # A_b1 stream (scalar ring), combine+out chasing per quarter
            for mt in range(NMT):
                emit_a_tile(1, mt, nc.scalar)
                emit_degas_transpose(mt)
                emit_combine_out(mt)

    nc.compile()
    return nc


def _get_bass():
    if "nc" not in _CACHE:
        _CACHE["nc"] = _build_bass()
    return _CACHE["nc"]


def _host_consts(kappa, alpha, w1, b1, w2, b2, pw, pb):
    kappa = float(np.asarray(kappa))
    alpha = float(np.asarray(alpha))
    w1 = np.asarray(w1, np.float32).reshape(16, 1)
    b1 = np.asarray(b1, np.float32).reshape(16)
    w2 = np.asarray(w2, np.float32).reshape(1, 16)
    b2 = np.asarray(b2, np.float32).reshape(1)
    pw = np.asarray(pw, np.float32).reshape(OUT_CH)
    pb = np.asarray(pb, np.float32).reshape(OUT_CH)

    kDT = DT * float(np.log1p(np.exp(kappa)))  # DT * softplus(kappa)

    ones_bd = np.zeros((128, C), np.float32)
    for f in range(2):
        for c in range(C):
            ones_bd[f * C + c, c] = 1.0

    w1r = np.tile(w1.T.astype(np.float32), (128, 1))
    b1r = np.tile(b1[None, :], (128, 1)).astype(np.float32)
    w2r_dt = np.tile((DT * w2).astype(np.float32), (128, 1))

    cvec = np.zeros((128, 4), np.float32)
    cvec[:, 0] = -kDT
    cvec[:, 1] = kDT
    cvec[:, 2] = DT * alpha
    cvec[:, 3] = DT * b2[0] - float(w2r_dt[0].sum())

    pwpb = np.zeros((128, 2 * OUT_CH), np.float32)
    pwpb[:, 0::2] = pw[None, :]
    pwpb[:, 1::2] = pb[None, :]

    idn = np.eye(128, dtype=np.float32)
    return ones_bd, w1r, b1r, w2r_dt, cvec, pwpb, idn


def kernel(x, A, phys_prior, kappa, alpha, w1, b1, w2, b2, pw, pb):
    x = np.ascontiguousarray(np.asarray(x, np.float32))
    A = np.ascontiguousarray(np.asarray(A, np.float32))
    phys_prior = np.ascontiguousarray(np.asarray(phys_prior, np.float32))
    ones_bd, w1r, b1r, w2r_dt, cvec, pwpb, idn = _host_consts(
        kappa, alpha, w1, b1, w2, b2, pw, pb
    )

    nc = _get_bass()
    core_ids = list(range(N_CORES))
    in_maps = []
    for i in core_ids:
        sl = slice(i * B_LOC, (i + 1) * B_LOC)
        in_maps.append(
            {
                "x_sh": x[sl],
                "a_sh": A[sl],
                "pp_sh": phys_prior[sl],
                "ones_bd": ones_bd,
                "w1r": w1r,
                "b1r": b1r,
                "w2r": w2r_dt,
                "cvec": cvec,
                "pwpb": pwpb,
                "idn": idn,
            }
        )

    res = run_bass_kernel_spmd(nc, in_maps, core_ids)
    out = np.concatenate([res.results[i]["out"] for i in range(N_CORES)], axis=0)
    return out.astype(np.float32)


if __name__ == "__main__":
    rng = np.random.default_rng(0)
    inputs = dict(
        x=rng.standard_normal((B, F_DIM, C, M)).astype(np.float32),
        A=rng.random((B, M, C, C)).astype(np.float32),
        phys_prior=rng.standard_normal((B, C, M)).astype(np.float32),
        kappa=np.float32(0.1),
        alpha=np.float32(0.05),
        w1=rng.standard_normal((16, 1)).astype(np.float32),
        b1=np.zeros(16, np.float32),
        w2=(rng.standard_normal((1, 16)) * 0.25).astype(np.float32),
        b2=np.zeros(1, np.float32),
        pw=rng.standard_normal(OUT_CH).astype(np.float32),
        pb=np.zeros(OUT_CH, np.float32),
    )
    out = kernel(**inputs)
    print("out", out.shape, out.dtype)


# revision 18
# speedup vs baseline: 1.0885x; 1.0885x over previous
"""Trainium2 Bass kernel for nn_DiffusionLayer (gnn_message_passing).

Computation (full shapes):
  x (16,64,64,512), A (16,512,64,64), phys_prior (16,64,512) ->
  corr (16,32,64,512)

Sharding: pure data parallel over batch B=16 across 8 cores (B_LOC=2 each).
All reductions are local to a (b, m) tile; scalar params replicated.

Per-core design (v2 — contiguous-A):
  A is loaded CONTIGUOUSLY as [p=m (128/tile), f=(c,d)=4096] so every
  partition reads one 16 KiB HBM run (line rate; the old (m0,c)-partition
  layout had 256 B runs -> ~1.6x DMA penalty, and its per-4-m fp32
  LDWEIGHTS burned ~120 us of PE).

  deg[m,c] = sum_d A[m,c,d]   : DVE tensor_reduce over innermost d.
  As[m,d]  = sum_c A[m,c,d]*sT[m,c] : gpsimd broadcast-multiply
             (stride-0 AP on sT over d) into prod, then DVE strided
             reduce over c (view [p, d, c]).  Split into two c-halves
             per tile so the reduce of half 0 overlaps the multiply of
             half 1.
  sT[m,c] (s transposed) via PE transposes of s_t chunks -> PSUM -> SBUF.
  deg_m/As_m [m,c-ish] are PE-transposed into PSUM tiles deg_ps/as_ps
  [p=(b,c), m] that feed the combine stage directly.

  s = mean_f x : PE blockdiag-ones matmul (as before), per b.
  combine + 1x1-conv output stages: unchanged from baseline (full-width
  [128=(b,c), m-quarter] DVE/ACT ops; out DMA 512 B runs at line rate).

DMA schedule: sync ring = x_b0(half), A_b0, A_b1(mt0,mt2); scalar ring =
consts, pp, x_b0(half), x_b1, A_b1(mt1,mt3), out.  x_b0 gets both rings
first (s_b0 gates the gpsimd product stream), A_b0 follows immediately,
x_b1 rides the scalar ring so sT_b1 is ready before the b1 product pass.
"""

import sys
import numpy as np

sys.path.insert(0, "/opt/trn_rl_repo")

import concourse.bass as bass  # noqa: E402
from concourse import bacc  # noqa: E402
import concourse.tile as tile  # noqa: E402
from concourse import mybir  # noqa: E402
from concourse.bass import broadcast_tensor_aps  # noqa: E402
from concourse.bass_utils import run_bass_kernel_spmd  # noqa: E402

B, F_DIM, C, M = 16, 64, 64, 512
OUT_CH = 32
DT = 1.0
N_CORES = 8
B_LOC = B // N_CORES  # 2
F32 = mybir.dt.float32
M_T = 128  # m's per A tile (= one quarter)
NMT = M // M_T  # 4 A tiles per b

_CACHE = {}


def _build_bass():
    nc = bacc.Bacc()

    x_sh = nc.declare_dram_parameter("x_sh", [B_LOC, F_DIM, C, M], F32, isOutput=False)
    a_sh = nc.declare_dram_parameter("a_sh", [B_LOC, M, C, C], F32, isOutput=False)
    pp_sh = nc.declare_dram_parameter("pp_sh", [B_LOC, C, M], F32, isOutput=False)
    ones_bd = nc.declare_dram_parameter("ones_bd", [128, C], F32, isOutput=False)
    w1r = nc.declare_dram_parameter("w1r", [128, 16], F32, isOutput=False)
    b1r = nc.declare_dram_parameter("b1r", [128, 16], F32, isOutput=False)
    w2r = nc.declare_dram_parameter("w2r", [128, 16], F32, isOutput=False)
    cvec = nc.declare_dram_parameter("cvec", [128, 4], F32, isOutput=False)
    pwpb = nc.declare_dram_parameter("pwpb", [128, 2 * OUT_CH], F32, isOutput=False)
    idn = nc.declare_dram_parameter("idn", [128, 128], F32, isOutput=False)
    out_sh = nc.declare_dram_parameter("out", [B_LOC, OUT_CH, C, M], F32, isOutput=True)

    AX = mybir.AxisListType
    OP = mybir.AluOpType
    ACTF = mybir.ActivationFunctionType

    with tile.TileContext(nc) as tc:
        with (
            tc.tile_pool(name="const", bufs=1) as cpool,
            tc.tile_pool(name="xp", bufs=4) as xpool,
            tc.tile_pool(name="ap", bufs=4) as apool,
            tc.tile_pool(name="pp", bufs=3) as prpool,
            tc.tile_pool(name="sp", bufs=1) as spool,
            tc.tile_pool(name="dm", bufs=4) as dmpool,
            tc.tile_pool(name="tmp", bufs=2) as tpool,
            tc.tile_pool(name="small", bufs=1) as smpool,
            tc.tile_pool(name="op", bufs=4) as opool,
            tc.tile_pool(name="ps_s", bufs=1, space="PSUM") as ps_s_pool,
            tc.tile_pool(name="ps_st", bufs=1, space="PSUM") as ps_st_pool,
            tc.tile_pool(name="ps_deg", bufs=1, space="PSUM") as ps_deg_pool,
            tc.tile_pool(name="ps_as", bufs=1, space="PSUM") as ps_as_pool,
        ):
            # ---- constants on the scalar ring ----
            NCC = C + 16 * 3 + 4 + 2 * OUT_CH
            call_t = cpool.tile([128, NCC], F32)
            nc.scalar.dma_start(call_t[:, 0:C], ones_bd[:])
            nc.scalar.dma_start(call_t[:, C : C + 16], w1r[:])
            nc.scalar.dma_start(call_t[:, C + 16 : C + 32], b1r[:])
            nc.scalar.dma_start(call_t[:, C + 32 : C + 48], w2r[:])
            nc.scalar.dma_start(call_t[:, C + 48 : C + 52], cvec[:])
            nc.scalar.dma_start(call_t[:, C + 52 : NCC], pwpb[:])
            ones_t = call_t[:, 0:C]
            w1r_t = call_t[:, C : C + 16]
            b1r_t = call_t[:, C + 16 : C + 32]
            w2r_t = call_t[:, C + 32 : C + 48]
            cvec_t = call_t[:, C + 48 : C + 52]
            pwpb_t = call_t[:, C + 52 : NCC]
            idn_t = cpool.tile([128, 128], F32)
            nc.scalar.dma_start(idn_t[:], idn[:])
            pp_t = spool.tile([128, M], F32)

            s_ps = ps_s_pool.tile([128, M], F32)
            s_t = spool.tile([128, M], F32)
            sT_ps = ps_st_pool.tile([128, 2 * NMT * C], F32)  # [128, 512]
            sT_sb = spool.tile([128, 2 * NMT * C], F32)
            deg_ps = ps_deg_pool.tile([128, M], F32)
            as_ps = ps_as_pool.tile([128, M], F32)
            snew = spool.tile([128, M], F32)

            NFP = F_DIM // 2  # 32 f-pairs per b
            FPG = 8  # f-pairs per DMA -> 4 x-tiles of 2 MiB per b
            NXG = NFP // FPG  # 4

            x_tiles = {}

            def emit_x_dma(b, g, eng):
                xt = xpool.tile([128, FPG * M], F32, tag="xt")
                xin = x_sh[b, 2 * g * FPG : 2 * (g + 1) * FPG].rearrange(
                    "(fp ftwo) c m -> ftwo c fp m", ftwo=2
                )
                eng.dma_start(xt[:].rearrange("p (fp m) -> p fp m", m=M), xin)
                x_tiles[(b, g)] = xt

            def emit_x_mm(b, g):
                xt = x_tiles.pop((b, g))
                for j in range(FPG):
                    fp = g * FPG + j
                    nc.tensor.matmul(
                        s_ps[b * C : (b + 1) * C, :],
                        ones_t[:],
                        xt[:, j * M : (j + 1) * M],
                        start=(fp == 0),
                        stop=(fp == NFP - 1),
                    )

            def emit_s_finish(b):
                # s_t = psum/F ; sT chunks via PE transpose -> psum -> SBUF.
                # Copies ride DVE: the ACT queue dispatches ring DMAs and can
                # jam behind their WAR waits.
                bsl = slice(b * C, (b + 1) * C)
                nc.vector.tensor_scalar_mul(s_t[bsl, :], s_ps[bsl, :], 1.0 / F_DIM)
                for ch in range(NMT):
                    co = b * NMT * C + ch * C
                    nc.tensor.transpose(
                        sT_ps[:, co : co + C],
                        s_t[bsl, ch * M_T : (ch + 1) * M_T],
                        idn_t[bsl, b * C : (b + 1) * C],
                    )
                half = slice(b * NMT * C, (b + 1) * NMT * C)
                nc.vector.tensor_copy(sT_sb[:, half], sT_ps[:, half])

            rdt_box = {}

            def emit_mlp():
                rsum = smpool.tile([128, 1], F32)
                nc.vector.tensor_reduce(rsum[:], s_t[:], axis=AX.X, op=OP.add)
                rin = smpool.tile([128, 1], F32)
                nc.vector.tensor_scalar_mul(rin[:], rsum[:], 1.0 / M)
                hp = smpool.tile([128, 16], F32)
                nc.vector.tensor_scalar(hp[:], w1r_t[:], rin[:], None, op0=OP.mult)
                nc.vector.tensor_add(hp[:], hp[:], b1r_t[:])
                hneg = smpool.tile([128, 16], F32)
                nc.vector.tensor_scalar_min(hneg[:], hp[:], 0.0)
                hexp = smpool.tile([128, 16], F32)
                nc.scalar.activation(hexp[:], hneg[:], ACTF.Exp)
                hrelu = smpool.tile([128, 16], F32)
                nc.vector.tensor_scalar_max(hrelu[:], hp[:], 0.0)
                helu = smpool.tile([128, 16], F32)
                nc.vector.tensor_add(helu[:], hexp[:], hrelu[:])
                # helu = elu + 1 ; host folds the -1 via cvec[:,3]
                hw = smpool.tile([128, 16], F32)
                nc.vector.tensor_mul(hw[:], helu[:], w2r_t[:])
                rpre = smpool.tile([128, 1], F32)
                nc.vector.tensor_reduce(rpre[:], hw[:], axis=AX.X, op=OP.add)
                rdt = smpool.tile([128, 1], F32)
                nc.vector.tensor_scalar(
                    rdt[:], rpre[:], cvec_t[:, 3:4], None, op0=OP.add
                )
                rdt_box["rdt"] = rdt

            CH = C // 2  # c-half = 32
            HW = CH * C  # flat cols per c-half = 2048

            # per-mt [p=m, f=(b,c)] staging tiles; one 128x128 transpose per
            # mt moves BOTH b's halves to [(b,c), m] psum (verifier requires
            # transpose outputs at psum partition 0).
            degm_tiles = {}
            asm_tiles = {}

            def emit_a_tile(b, mt, eng):
                """Stream one A tile and produce deg/As halves."""
                at = apool.tile([128, C * C], F32, tag="at")
                ain = a_sh[b, mt * M_T : (mt + 1) * M_T].rearrange("m c d -> m (c d)")
                eng.dma_start(at[:], ain)
                if mt not in degm_tiles:
                    degm_tiles[mt] = dmpool.tile(
                        [128, 2 * C], F32, tag=f"degm{mt}", name=f"degm{mt}"
                    )
                    asm_tiles[mt] = dmpool.tile(
                        [128, 2 * C], F32, tag=f"asm{mt}", name=f"asm{mt}"
                    )
                degm = degm_tiles[mt]
                asm = asm_tiles[mt]
                bh = slice(b * C, (b + 1) * C)  # column half for this b
                # deg_m[m, c] = sum_d A[m, c, d]
                nc.vector.tensor_reduce(
                    degm[:, bh], at[:].rearrange("p (c d) -> p c d", d=C),
                    axis=AX.X, op=OP.add,
                )
                # As: two c-halves; gpsimd broadcast multiply, DVE strided reduce
                co = b * NMT * C + mt * C
                asp = []
                for h in range(2):
                    prod = prpool.tile([128, HW], F32, tag="prod")
                    a_v = at[:, h * HW : (h + 1) * HW].rearrange(
                        "p (c d) -> p c d", d=C
                    )
                    st_v = sT_sb[:, co + h * CH : co + (h + 1) * CH].rearrange(
                        "p (c one) -> p c one", one=1
                    )
                    a_bv, st_bv = broadcast_tensor_aps(a_v, st_v)
                    # gpsimd walks APs in software: write prod TRANSPOSED
                    # ((d, c) layout) so the DVE reduce below is contiguous
                    # (strided reduce measured ~2x slower).
                    nc.gpsimd.tensor_tensor(
                        prod[:].rearrange("p (d c) -> p c d", c=CH),
                        a_bv, st_bv, op=OP.mult,
                    )
                    ph = dmpool.tile([128, C], F32, tag=f"asp{h}")
                    nc.vector.tensor_reduce(
                        ph[:], prod[:].rearrange("p (d c) -> p d c", c=CH),
                        axis=AX.X, op=OP.add,
                    )
                    asp.append(ph)
                nc.vector.tensor_add(asm[:, bh], asp[0][:], asp[1][:])

            def emit_degas_transpose(mt):
                nc.tensor.transpose(
                    deg_ps[:, mt * M_T : (mt + 1) * M_T],
                    degm_tiles.pop(mt)[:], idn_t[:],
                )
                nc.tensor.transpose(
                    as_ps[:, mt * M_T : (mt + 1) * M_T],
                    asm_tiles.pop(mt)[:], idn_t[:],
                )

            OG = 8  # out channels per DMA

            def emit_combine_out(q):
                hs = slice(q * M_T, (q + 1) * M_T)
                t2p = tpool.tile([128, M_T], F32, tag="t2p")
                nc.vector.tensor_scalar(
                    t2p[:], deg_ps[:, hs], cvec_t[:, 0:1], 1.0,
                    op0=OP.mult, op1=OP.add,
                )
                t2 = tpool.tile([128, M_T], F32, tag="t2")
                nc.vector.tensor_mul(t2[:], t2p[:], s_t[:, hs])
                t3 = tpool.tile([128, M_T], F32, tag="t3")
                nc.vector.tensor_scalar(
                    t3[:], as_ps[:, hs], cvec_t[:, 1:2], None, op0=OP.mult
                )
                t4 = tpool.tile([128, M_T], F32, tag="t4")
                nc.vector.tensor_add(t4[:], t2[:], t3[:])
                t5 = tpool.tile([128, M_T], F32, tag="t5")
                nc.vector.tensor_scalar(
                    t5[:], pp_t[:, hs], cvec_t[:, 2:3], rdt_box["rdt"][:],
                    op0=OP.mult, op1=OP.add,
                )
                nc.vector.tensor_add(snew[:, hs], t4[:], t5[:])
                for og in range(OUT_CH // OG):
                    ot = opool.tile([128, OG * M_T], F32, tag="ot")
                    for g in range(OG):
                        o = og * OG + g
                        if g % 2 == 0:
                            nc.vector.tensor_scalar(
                                ot[:, g * M_T : (g + 1) * M_T],
                                snew[:, hs],
                                pwpb_t[:, 2 * o : 2 * o + 1],
                                pwpb_t[:, 2 * o + 1 : 2 * o + 2],
                                op0=OP.mult, op1=OP.add,
                            )
                        else:
                            nc.scalar.activation(
                                ot[:, g * M_T : (g + 1) * M_T],
                                snew[:, hs],
                                ACTF.Identity,
                                bias=pwpb_t[:, 2 * o + 1 : 2 * o + 2],
                                scale=pwpb_t[:, 2 * o : 2 * o + 1],
                            )
                    for b in range(B_LOC):
                        odst = out_sh[
                            b, og * OG : (og + 1) * OG, :, q * M_T : (q + 1) * M_T
                        ].rearrange("o c m -> c o m")
                        osrc = ot[b * C : (b + 1) * C, :].rearrange(
                            "p (o m) -> p o m", m=M_T
                        )
                        oeng = nc.sync if (og + b) % 2 == 0 else nc.scalar
                        oeng.dma_start(odst, osrc)

            # ---------------- emission order ----------------
            # Both rings carry interleaved x/A so tiles land evenly:
            #   sync:   xb0g0 xb0g2 | Ab0mt0 xb1g0 Ab0mt2 xb1g2 | Ab1mt0 Ab1mt2 | outs
            #   scalar: consts xb0g1 xb0g3 | Ab0mt1 xb1g1 Ab0mt3 xb1g3 pp | Ab1mt1 Ab1mt3 | outs
            emit_x_dma(0, 0, nc.sync)
            emit_x_dma(0, 1, nc.scalar)
            emit_x_dma(0, 2, nc.sync)
            emit_x_dma(0, 3, nc.scalar)
            for g in range(NXG):
                emit_x_mm(0, g)
            emit_s_finish(0)
            # A_b0 stream with x_b1 DMAs and s_b1 matmul groups interleaved
            for mt in range(NMT):
                emit_a_tile(0, mt, nc.sync if mt % 2 == 0 else nc.scalar)
                emit_x_dma(1, mt, nc.sync if mt % 2 == 0 else nc.scalar)
                emit_x_mm(1, mt)
            nc.scalar.dma_start(pp_t[:], pp_sh[:])
            emit_s_finish(1)
            emit_mlp()
            # A_b1 stream (rings alternated), combine+out chasing per quarter
            for mt in range(NMT):
                emit_a_tile(1, mt, nc.sync if mt % 2 == 0 else nc.scalar)
                emit_degas_transpose(mt)
                emit_combine_out(mt)

    nc.compile()
    return nc


def _get_bass():
    if "nc" not in _CACHE:
        _CACHE["nc"] = _build_bass()
    return _CACHE["nc"]


def _host_consts(kappa, alpha, w1, b1, w2, b2, pw, pb):
    kappa = float(np.asarray(kappa))
    alpha = float(np.asarray(alpha))
    w1 = np.asarray(w1, np.float32).reshape(16, 1)
    b1 = np.asarray(b1, np.float32).reshape(16)
    w2 = np.asarray(w2, np.float32).reshape(1, 16)
    b2 = np.asarray(b2, np.float32).reshape(1)
    pw = np.asarray(pw, np.float32).reshape(OUT_CH)
    pb = np.asarray(pb, np.float32).reshape(OUT_CH)

    kDT = DT * float(np.log1p(np.exp(kappa)))  # DT * softplus(kappa)

    ones_bd = np.zeros((128, C), np.float32)
    for f in range(2):
        for c in range(C):
            ones_bd[f * C + c, c] = 1.0

    w1r = np.tile(w1.T.astype(np.float32), (128, 1))
    b1r = np.tile(b1[None, :], (128, 1)).astype(np.float32)
    w2r_dt = np.tile((DT * w2).astype(np.float32), (128, 1))

    cvec = np.zeros((128, 4), np.float32)
    cvec[:, 0] = -kDT
    cvec[:, 1] = kDT
    cvec[:, 2] = DT * alpha
    cvec[:, 3] = DT * b2[0] - float(w2r_dt[0].sum())

    pwpb = np.zeros((128, 2 * OUT_CH), np.float32)
    pwpb[:, 0::2] = pw[None, :]
    pwpb[:, 1::2] = pb[None, :]

    idn = np.eye(128, dtype=np.float32)
    return ones_bd, w1r, b1r, w2r_dt, cvec, pwpb, idn


def kernel(x, A, phys_prior, kappa, alpha, w1, b1, w2, b2, pw, pb):
    x = np.ascontiguousarray(np.asarray(x, np.float32))
    A = np.ascontiguousarray(np.asarray(A, np.float32))
    phys_prior = np.ascontiguousarray(np.asarray(phys_prior, np.float32))
    ones_bd, w1r, b1r, w2r_dt, cvec, pwpb, idn = _host_consts(
        kappa, alpha, w1, b1, w2, b2, pw, pb
    )

    nc = _get_bass()
    core_ids = list(range(N_CORES))
    in_maps = []
    for i in core_ids:
        sl = slice(i * B_LOC, (i + 1) * B_LOC)
        in_maps.append(
            {
                "x_sh": x[sl],
                "a_sh": A[sl],
                "pp_sh": phys_prior[sl],
                "ones_bd": ones_bd,
                "w1r": w1r,
                "b1r": b1r,
                "w2r": w2r_dt,
                "cvec": cvec,
                "pwpb": pwpb,
                "idn": idn,
            }
        )

    res = run_bass_kernel_spmd(nc, in_maps, core_ids)
    out = np.concatenate([res.results[i]["out"] for i in range(N_CORES)], axis=0)
    return out.astype(np.float32)


if __name__ == "__main__":
    rng = np.random.default_rng(0)
    inputs = dict(
        x=rng.standard_normal((B, F_DIM, C, M)).astype(np.float32),
        A=rng.random((B, M, C, C)).astype(np.float32),
        phys_prior=rng.standard_normal((B, C, M)).astype(np.float32),
        kappa=np.float32(0.1),
        alpha=np.float32(0.05),
        w1=rng.standard_normal((16, 1)).astype(np.float32),
        b1=np.zeros(16, np.float32),
        w2=(rng.standard_normal((1, 16)) * 0.25).astype(np.float32),
        b2=np.zeros(1, np.float32),
        pw=rng.standard_normal(OUT_CH).astype(np.float32),
        pb=np.zeros(OUT_CH, np.float32),
    )
    out = kernel(**inputs)
    print("out", out.shape, out.dtype)
